# revision 38
# baseline (speedup 1.0000x reference)
"""FP8Linear on 8 Trainium2 NeuronCores (Bass/Tile, SPMD) -- v3 "4x2 grid".

Reference math (per nn_FP8Linear):
    amax_x = max|x|, amax_w = max|w|               (global)
    x_scale = amax_x / C,  w_scale = max(amax_x, amax_w) / C,  C = fp32(448*0.8)
    out = dequant(e4m3fn(x/x_scale)) @ dequant(e4m3fn(w/w_scale)).T + bias

Sharding: 4 token-shards x 2 dout-shards. Per core: x shard 16MB (2048
tokens, split into an "amax half" read first and a streamed half), w shard
8MB (1024 out-features), out 2048x1024 written fp16 (host upcasts; adds
~3e-5 to the 2.8e-4 rel err, vs the 2e-2 gate).

Measured structure (best 166us; PE-bound behind a collective wall):
  t=0..45    P1 DMA: bias, w amax-slice (2MB = the core's own k-tiles, no
             duplicate read), x amax-half (8MB); DVE amax chases arrivals.
             x/w are k-tile-permuted identically on host so each core's
             first w k-tiles are its amax slice (k-perm leaves x@w.T
             invariant).  PE warm-up matmuls keep the clock boosted.
  t=50..85   AllGather [1,2]->[8,2].  The Comms/ncfw stack only becomes
             ready ~75us after kernel start (first-call wall, independent
             of issue time -- a t=0 dummy collective does NOT dodge it, it
             just queues ahead; remote_dma_broadcast bypasses ncfw and cut
             this to ~132us total but delivered corrupted payloads, see
             transcript).  P2 (w rest) + P3 (x other half) DMA is gated on
             the collective: releasing it early lets fast cores starve the
             laggard's P1 (chip-shared DMA bandwidth) and delays the
             collective more than the prefetch saves (measured +7us).
  t=85..94   download, partition-fold, scales (5 DVE ops), first quant
             chunks: (wq, xq_a) pairs kk 0-4 on DVE, 5-7 on ACT, so the
             PE's k-ordered chase sees pairs at the combined rate.
  t=94..163  256 DoubleRow fp8 matmuls, fully saturated at ~263ns each
             (1024 moving rows @ 2.4GHz + 13% DR adder; LDWEIGHTS is
             pipelined under the previous matmul).  m0..7 run as two
             4-m-group / 8-PSUM-bank blocks chasing the quant stream;
             m8..15 are each gated only on their own 1MB x_b block.
             Drains (scale+bias, fp16) on DVE; out DMA on the ACT ring
             (sync HWDGE ring carries only the ungated input stream --
             a sem-gated entry there head-of-line blocks everything).

Hard-won constraints (do not regress):
  - gpsimd bulk tensor ops are ~8.7 Gelem/s Q7 software loops AND stall
    DVE via shared SBUF ports; keep gpsimd to ISA ops (partition
    reduce/broadcast, collective) only.
  - fp8e4 max is +-240 (not e4m3fn's 448): quantize t = x * rn(1/(2s));
    halving exactly preserves the e4m3fn grid and 4*s_x*s_w undoes it.
  - DMA engine bandwidth is chip-shared: per-core DMA priority requires
    phase-gating, not just ring order.
"""
import numpy as np
from contextlib import ExitStack

import concourse.bacc as bacc
import concourse.bass as bass
import concourse.bass_isa as bass_isa
import concourse.tile as tile
from concourse import mybir
from concourse.bass_utils import run_bass_kernel_spmd
from concourse.tile_rust import add_dep_helper
import bass_rust

N_CORES = 8
B, S, DIN, DOUT = 4, 2048, 2048, 2048
TOK = B * S                  # 8192 tokens
GT, GN = 4, 2                # grid: 4 token-shards x 2 dout-shards
TSH = TOK // GT              # 2048 tokens per core
DSH = DOUT // GN             # 1024 out-features per core
KT = DIN // 128              # 16 k-tiles
HT = TSH // 2                # 1024 tokens per half
MT = TSH // 128              # 16 m-groups per core
NB = HT // 128               # 8 x_b blocks

C_F64 = 448.0 * 0.8
RC = float(np.float32(1.0 / np.float64(np.float32(C_F64))))  # rn(1/C)

F32 = mybir.dt.float32
F16 = mybir.dt.float16
F8 = mybir.dt.float8e4

_built = None


def _build():
    global _built
    if _built is not None:
        return _built
    nc = bacc.Bacc("TRN2", target_bir_lowering=False, num_devices=N_CORES)

    xa_d = nc.dram_tensor("xa", [128, KT * HT], F32, kind="ExternalInput")
    xb_d = nc.dram_tensor("xb", [NB * 128, KT * 128], F32, kind="ExternalInput")
    ws_d = nc.dram_tensor("ws", [128, KT * DSH], F32, kind="ExternalInput")
    bias_d = nc.dram_tensor("bias", [1, DSH], F32, kind="ExternalInput")
    out_d = nc.dram_tensor("out", [TSH, DSH], F16, kind="ExternalOutput")


    WSLK = 4                 # k-tiles in this core's w amax slice
    NXC = 8                  # x_a DMA/amax chunks (2 k-tiles = 1MB each)
    NWC = 6                  # w-rest DMA chunks (2 k-tiles = 1MB each)

    with tile.TileContext(nc) as tc, ExitStack() as ctx:
        pool = ctx.enter_context(tc.tile_pool(name="pool", bufs=1))
        xbpool = ctx.enter_context(tc.tile_pool(name="xbpool", bufs=2))
        opool = ctx.enter_context(tc.tile_pool(name="opool", bufs=2))
        psum = ctx.enter_context(tc.tile_pool(name="psum", bufs=8, space="PSUM"))
        dram = ctx.enter_context(tc.tile_pool(name="dram", bufs=1, space="DRAM"))

        xa_sb = pool.tile([128, KT, HT], F32)       # 64KB/part
        ws_sb = pool.tile([128, KT, DSH], F32)      # 64KB/part
        xq = pool.tile([128, KT, TSH], F8)          # 32KB/part (a|b halves)
        wq = pool.tile([128, KT, DSH], F8)          # 16KB/part
        bias_bc = pool.tile([128, DSH], F16)        # 2KB/part
        bias_row = pool.tile([1, DSH], F32)

        # ---- sync ring: input stream in 3 sync-gated phases.  DMA engine
        # bandwidth is CHIP-shared across the 8 cores; without gating, the
        # other cores' bulk streams starve this core's amax-critical reads
        # and the collective waits on the slowest peer.
        with nc.named_scope("dma_in"):
            nc.sync.dma_start(bias_row[:], bias_d[:])
            xa_view = xa_d[:].rearrange("p (k c) -> p k c", k=KT)
            ws_view = ws_d[:].rearrange("p (k c) -> p k c", k=KT)
            # P1: amax coverage (w slice + x half), 1-ktile chunks
            for c in range(WSLK):
                nc.sync.dma_start(ws_sb[:, c:c + 1, :], ws_view[:, c:c + 1, :])
            xa_dmas = []
            for c in range(KT):
                xa_dmas.append(nc.sync.dma_start(xa_sb[:, c:c + 1, :],
                                                 xa_view[:, c:c + 1, :]))
            # P2/P3 are emitted below, gated on the collective: releasing
            # them on local-P1-complete lets the fastest core's bulk traffic
            # starve the slowest core's P1 (chip-shared DMA bandwidth), which
            # amplifies skew the collective then waits out.

        # ---- local abs-max on DVE, chasing the arrivals (gpsimd bulk ops
        # are slow Q7 software loops AND poison DVE via shared SBUF ports)
        red = pool.tile([128, WSLK + KT], F32)
        am2 = pool.tile([128, 2], F32)
        zfold = pool.tile([32, 32], F32)
        gfold = pool.tile([32, 32], F32)
        nc.vector.memset(zfold[:], 0.0)
        nc.vector.memset(gfold[:], 0.0)
        with nc.named_scope("amax"):
            for c in range(WSLK):
                nc.vector.tensor_reduce(red[:, c:c + 1], ws_sb[:, c:c + 1, :],
                                        mybir.AxisListType.XY,
                                        mybir.AluOpType.max,
                                        apply_absolute_value=True)
            for c in range(KT):
                nc.vector.tensor_reduce(red[:, WSLK + c:WSLK + c + 1],
                                        xa_sb[:, c:c + 1, :],
                                        mybir.AxisListType.XY,
                                        mybir.AluOpType.max,
                                        apply_absolute_value=True)
            nc.vector.tensor_reduce(am2[:, 1:2], red[:, 0:WSLK],
                                    mybir.AxisListType.X, mybir.AluOpType.max)
            nc.vector.tensor_reduce(am2[:, 0:1], red[:, WSLK:],
                                    mybir.AxisListType.X, mybir.AluOpType.max)

        # ---- global amax: AllGather [1,2]->[8,2] (small DMAs on ACT ring)
        with nc.named_scope("cc_amax"):
            amg_loc = pool.tile([128, 2], F32)
            nc.gpsimd.partition_all_reduce(amg_loc[:], am2[:], 128,
                                           bass_isa.ReduceOp.max)
            cc_in = dram.tile([1, 2], F32)
            cc_out = dram.tile([8, 2], F32)
            ag8 = pool.tile([8, 2], F32)
            nc.scalar.dma_start(cc_in[:], amg_loc[0:1, :])
            cc_inst = nc.gpsimd.collective_compute(
                "AllGather", mybir.AluOpType.bypass,
                replica_groups=[list(range(N_CORES))],
                ins=[cc_in[:].opt()], outs=[cc_out[:].opt()])
            ag_dma = nc.scalar.dma_start(ag8[:], cc_out[:])
            # fold the 8 rows on DVE (32x32 block transposes) -- keeps the
            # post-collective chain off gpsimd except one broadcast
            nc.vector.tensor_copy(zfold[0:8, 0:2], ag8[:])
            tfold = pool.tile([32, 32], F32)
            nc.vector.transpose(tfold[:], zfold[:])
            nc.vector.tensor_reduce(gfold[0:2, 0:1], tfold[0:2, :],
                                    mybir.AxisListType.X, mybir.AluOpType.max)
            t2f = pool.tile([32, 32], F32)
            nc.vector.transpose(t2f[:], gfold[:])   # row0 = [ax, aw]
            agb = pool.tile([128, 2], F32)
            nc.gpsimd.partition_broadcast(agb[:], t2f[0:1, 0:2])

        # ---- P2 (w rest) then P3 (x other half): with the early barrier
        # absorbing start skew, all cores finish P1 together, so gating on
        # local-P1 no longer lets fast cores starve the laggard's P1
        with nc.named_scope("dma_late"):
            p2 = []
            for c in range(NWC):
                k0 = WSLK + 2 * c
                d = nc.sync.dma_start(ws_sb[:, k0:k0 + 2, :],
                                      ws_view[:, k0:k0 + 2, :])
                add_dep_helper(d.ins, cc_inst.ins, sync=True,
                               reason="P2 after the collective barrier: "
                               "releasing bulk DMA early lets fast cores "
                               "starve the laggard's P1 (chip-shared DMA), "
                               "delaying the collective more than the "
                               "prefetch saves (measured +7us)")
                p2.append(d)
            xb_tiles = []
            for b in range(NB):
                xbt = xbpool.tile([128, KT, 128], F32, name="xbt")
                d = nc.sync.dma_start(xbt[:], xb_d[b * 128:(b + 1) * 128, :])
                add_dep_helper(d.ins, p2[-1].ins, sync=True,
                               reason="P3 after P2")
                xb_tiles.append(xbt)

        # ---- scales: rx = 1/(2 RC ax), rw = 1/(2 RC max(ax,aw)),
        # sc4 = 4 sx sw = (2 RC ax)(2 RC max) -- 5 DVE ops total
        scal = pool.tile([128, 8], F32)  # [d2x, d2w, rx, rw, sc4, ...]
        with nc.named_scope("scales"):
            t01 = pool.tile([128, 2], F32)
            nc.vector.tensor_tensor(t01[:, 1:2], agb[:, 0:1], agb[:, 1:2],
                                    mybir.AluOpType.max)
            nc.vector.tensor_copy(t01[:, 0:1], agb[:, 0:1])
            nc.vector.tensor_scalar(scal[:, 0:2], t01[:], 2.0 * RC, None,
                                    mybir.AluOpType.mult)
            nc.vector.reciprocal(scal[:, 2:4], scal[:, 0:2])    # rx, rw
            nc.vector.tensor_tensor(scal[:, 4:5], scal[:, 0:1],
                                    scal[:, 1:2], mybir.AluOpType.mult)

        # bias broadcast (gpsimd, cheap, before xb quants)
        bias16 = pool.tile([1, DSH], F16)
        nc.vector.tensor_copy(bias16[:], bias_row[:])
        nc.gpsimd.partition_broadcast(bias_bc[:], bias16[:])

        # ---- PE warm-up: junk matmuls gated on P1-DMA-complete keep the PE
        # clock boosted (HAM) through the collective window so real matmuls
        # start at full rate instead of ramping for 3us
        warm_l = pool.tile([128, 2, 128], F8)
        warm_r = pool.tile([128, 2, 512], F8)
        nc.vector.memset(warm_l[:], 0.0)
        nc.vector.memset(warm_r[:], 0.0)
        warm_ps = psum.tile([128, 512], F32, name="ps")
        for i in range(16):
            w_mm = nc.tensor.matmul(warm_ps[:], warm_l[:], warm_r[:],
                                    start=True, stop=True,
                                    perf_mode=mybir.MatmulPerfMode.DoubleRow)
            if i == 0:
                add_dep_helper(w_mm.ins, ag_dma.ins, sync=True,
                               reason="warm PE right before real matmuls: "
                               "warming at P1-end decays during the ~40us "
                               "collective wall (ramp costs ~2.5us)")

        # ---- quantize (wq, xq_a) per k-chunk; pairs kk 0-4 on DVE
        # (1.1us/chunk), kk 5-7 on ACT (2.1us/chunk) so the PE's k-ordered
        # chase sees pairs at the combined rate
        with nc.named_scope("quant"):
            for c in range(KT // 2):
                if c < 5:
                    nc.vector.tensor_scalar(wq[:, 2 * c:2 * c + 2, :],
                                            ws_sb[:, 2 * c:2 * c + 2, :],
                                            scal[:, 3:4], None,
                                            mybir.AluOpType.mult)
                    nc.vector.tensor_scalar(xq[:, 2 * c:2 * c + 2, 0:HT],
                                            xa_sb[:, 2 * c:2 * c + 2, :],
                                            scal[:, 2:3], None,
                                            mybir.AluOpType.mult)
                else:
                    nc.scalar.activation(
                        wq[:, 2 * c:2 * c + 2, :],
                        ws_sb[:, 2 * c:2 * c + 2, :],
                        bass_rust.ActivationFunctionType.Copy,
                        bias=0.0, scale=scal[:, 3:4])
                    nc.scalar.activation(
                        xq[:, 2 * c:2 * c + 2, 0:HT],
                        xa_sb[:, 2 * c:2 * c + 2, :],
                        bass_rust.ActivationFunctionType.Copy,
                        bias=0.0, scale=scal[:, 2:3])

        # ---- matmuls: blocked quant-chasing schedule + per-block tail
        def mm_group(m, ptiles, xcols):
            for kk in range(KT // 2):
                for n in range(GN):
                    nc.tensor.matmul(
                        ptiles[n][:],
                        xq[:, 2 * kk:2 * kk + 2, xcols],
                        wq[:, 2 * kk:2 * kk + 2, n * 512:(n + 1) * 512],
                        start=(kk == 0), stop=(kk == KT // 2 - 1),
                        perf_mode=mybir.MatmulPerfMode.DoubleRow)

        def drain(m, ptiles):
            # last m-group: drain/DMA in 512-col halves for a shorter tail
            pieces = 2 if m == MT - 1 else 1
            osb = opool.tile([128, DSH], F16, name="osb")
            for n in range(GN):
                for p in range(pieces):
                    W = 512 // pieces
                    c0 = n * 512 + p * W
                    nc.vector.scalar_tensor_tensor(
                        osb[:, c0:c0 + W], ptiles[n][:, p * W:(p + 1) * W],
                        scal[:, 4:5], bias_bc[:, c0:c0 + W],
                        mybir.AluOpType.mult, mybir.AluOpType.add)
                    if pieces == 2:
                        nc.scalar.dma_start(
                            out_d[m * 128:(m + 1) * 128, c0:c0 + W],
                            osb[:, c0:c0 + W])
            if pieces == 1:
                nc.scalar.dma_start(out_d[m * 128:(m + 1) * 128, :], osb[:])

        with nc.named_scope("mm"):
            # m0..7 (x_a half): two 4-m-group blocks, k-chunk-major so the
            # PE chases the wq/xq_a quant streams chunk by chunk
            for blk in range(2):
                ptiles = [[psum.tile([128, 512], F32, name="ps")
                           for _ in range(GN)] for _ in range(4)]
                for kk in range(KT // 2):
                    for mi in range(4):
                        m = blk * 4 + mi
                        for n in range(GN):
                            nc.tensor.matmul(
                                ptiles[mi][n][:],
                                xq[:, 2 * kk:2 * kk + 2,
                                   m * 128:(m + 1) * 128],
                                wq[:, 2 * kk:2 * kk + 2,
                                   n * 512:(n + 1) * 512],
                                start=(kk == 0), stop=(kk == KT // 2 - 1),
                                perf_mode=mybir.MatmulPerfMode.DoubleRow)
                for mi in range(4):
                    drain(blk * 4 + mi, ptiles[mi])
            # m8..15: each gated only on its own arriving x_b block
            for b in range(NB):
                nc.scalar.activation(
                    xq[:, :, HT + b * 128:HT + (b + 1) * 128],
                    xb_tiles[b][:],
                    bass_rust.ActivationFunctionType.Copy,
                    bias=0.0, scale=scal[:, 2:3])
                m = 8 + b
                ptiles = [psum.tile([128, 512], F32, name="ps")
                          for _ in range(GN)]
                mm_group(m, ptiles, slice(m * 128, (m + 1) * 128))
                drain(m, ptiles)

    nc.compile()
    _built = nc
    return nc


def _pmaj(a, ktp):
    """[DIN, C] -> [128, KT*C] partition-major with k-tile permutation."""
    return np.ascontiguousarray(
        a.reshape(KT, 128, -1)[ktp].transpose(1, 0, 2).reshape(128, -1))


def stage_inputs(x, weight, bias):
    x = np.asarray(x, dtype=np.float32)
    weight = np.asarray(weight, dtype=np.float32)
    bias = np.asarray(bias, dtype=np.float32)
    x2 = x.reshape(TOK, DIN)
    wt = weight.T                                   # [DIN, DOUT]
    in_maps = []
    for i in range(N_CORES):
        t, n = divmod(i, GN)
        ktp = np.r_[np.arange(4 * t, 4 * t + 4),
                    [k for k in range(KT) if not 4 * t <= k < 4 * t + 4]]
        xs = x2[2048 * t:2048 * (t + 1)].T          # [DIN, 2048]
        xa = xs[:, HT * n:HT * (n + 1)]             # this core's amax half
        xb = xs[:, HT * (1 - n):HT * (2 - n)]
        xbt = xb.reshape(KT, 128, NB, 128)[ktp]     # [k,p,b,c]
        in_maps.append({
            "xa": _pmaj(xa, ktp),
            "xb": np.ascontiguousarray(
                xbt.transpose(2, 1, 0, 3).reshape(NB * 128, KT * 128)),
            "ws": _pmaj(wt[:, DSH * n:DSH * (n + 1)], ktp),
            "bias": np.ascontiguousarray(bias[DSH * n:DSH * (n + 1)]
                                         ).reshape(1, DSH),
        })
    return in_maps


def assemble(results):
    full = np.empty((TOK, DOUT), np.float32)
    for i, r in enumerate(results):
        t, n = divmod(i, GN)
        o = np.asarray(r["out"]).astype(np.float32)
        full[2048 * t + HT * n:2048 * t + HT * (n + 1),
             DSH * n:DSH * (n + 1)] = o[0:HT]
        full[2048 * t + HT * (1 - n):2048 * t + HT * (2 - n),
             DSH * n:DSH * (n + 1)] = o[HT:TSH]
    return full.reshape(B, S, DOUT)


def kernel(x, weight, bias):
    in_maps = stage_inputs(x, weight, bias)
    nc = _build()
    br = run_bass_kernel_spmd(nc, in_maps, list(range(N_CORES)))
    return assemble(br.results)


# revision 41
# speedup vs baseline: 1.0861x; 1.0861x over previous
"""FP8Linear on 8 Trainium2 NeuronCores (Bass/Tile, SPMD) -- v3 "4x2 grid".

Reference math (per nn_FP8Linear):
    amax_x = max|x|, amax_w = max|w|               (global)
    x_scale = amax_x / C,  w_scale = max(amax_x, amax_w) / C,  C = fp32(448*0.8)
    out = dequant(e4m3fn(x/x_scale)) @ dequant(e4m3fn(w/w_scale)).T + bias

Sharding: 4 token-shards x 2 dout-shards. Per core: x shard 16MB (2048
tokens, split into an "amax half" read first and a streamed half), w shard
8MB (1024 out-features), out 2048x1024 written fp16 (host upcasts; adds
~3e-5 to the 2.8e-4 rel err, vs the 2e-2 gate).

Measured structure (best 166us; PE-bound behind a collective wall):
  t=0..45    P1 DMA: bias, w amax-slice (2MB = the core's own k-tiles, no
             duplicate read), x amax-half (8MB); DVE amax chases arrivals.
             x/w are k-tile-permuted identically on host so each core's
             first w k-tiles are its amax slice (k-perm leaves x@w.T
             invariant).  PE warm-up matmuls keep the clock boosted.
  t=50..85   AllGather [1,2]->[8,2].  The Comms/ncfw stack only becomes
             ready ~75us after kernel start (first-call wall, independent
             of issue time -- a t=0 dummy collective does NOT dodge it, it
             just queues ahead; remote_dma_broadcast bypasses ncfw and cut
             this to ~132us total but delivered corrupted payloads, see
             transcript).  P2 (w rest) + P3 (x other half) DMA is gated on
             the collective: releasing it early lets fast cores starve the
             laggard's P1 (chip-shared DMA bandwidth) and delays the
             collective more than the prefetch saves (measured +7us).
  t=85..94   download, partition-fold, scales (5 DVE ops), first quant
             chunks: (wq, xq_a) pairs kk 0-4 on DVE, 5-7 on ACT, so the
             PE's k-ordered chase sees pairs at the combined rate.
  t=94..163  256 DoubleRow fp8 matmuls, fully saturated at ~263ns each
             (1024 moving rows @ 2.4GHz + 13% DR adder; LDWEIGHTS is
             pipelined under the previous matmul).  m0..7 run as two
             4-m-group / 8-PSUM-bank blocks chasing the quant stream;
             m8..15 are each gated only on their own 1MB x_b block.
             Drains (scale+bias, fp16) on DVE; out DMA on the ACT ring
             (sync HWDGE ring carries only the ungated input stream --
             a sem-gated entry there head-of-line blocks everything).

Hard-won constraints (do not regress):
  - gpsimd bulk tensor ops are ~8.7 Gelem/s Q7 software loops AND stall
    DVE via shared SBUF ports; keep gpsimd to ISA ops (partition
    reduce/broadcast, collective) only.
  - fp8e4 max is +-240 (not e4m3fn's 448): quantize t = x * rn(1/(2s));
    halving exactly preserves the e4m3fn grid and 4*s_x*s_w undoes it.
  - DMA engine bandwidth is chip-shared: per-core DMA priority requires
    phase-gating, not just ring order.
"""
import numpy as np
from contextlib import ExitStack

import concourse.bacc as bacc
import concourse.bass as bass
import concourse.bass_isa as bass_isa
import concourse.tile as tile
from concourse import mybir
from concourse.bass_utils import run_bass_kernel_spmd
from concourse.tile_rust import add_dep_helper
import bass_rust

N_CORES = 8
B, S, DIN, DOUT = 4, 2048, 2048, 2048
TOK = B * S                  # 8192 tokens
GT, GN = 4, 2                # grid: 4 token-shards x 2 dout-shards
TSH = TOK // GT              # 2048 tokens per core
DSH = DOUT // GN             # 1024 out-features per core
KT = DIN // 128              # 16 k-tiles
HT = TSH // 2                # 1024 tokens per half
MT = TSH // 128              # 16 m-groups per core
NB = HT // 128               # 8 x_b blocks

C_F64 = 448.0 * 0.8
RC = float(np.float32(1.0 / np.float64(np.float32(C_F64))))  # rn(1/C)

F32 = mybir.dt.float32
F16 = mybir.dt.float16
F8 = mybir.dt.float8e4

_built = None


def _build():
    global _built
    if _built is not None:
        return _built
    nc = bacc.Bacc("TRN2", target_bir_lowering=False, num_devices=N_CORES)

    xa_d = nc.dram_tensor("xa", [128, KT * HT], F32, kind="ExternalInput")
    xb_d = nc.dram_tensor("xb", [NB * 128, KT * 128], F32, kind="ExternalInput")
    ws_d = nc.dram_tensor("ws", [128, KT * DSH], F32, kind="ExternalInput")
    bias_d = nc.dram_tensor("bias", [1, DSH], F32, kind="ExternalInput")
    out_d = nc.dram_tensor("out", [TSH, DSH], F16, kind="ExternalOutput")


    WSLK = 4                 # k-tiles in this core's w amax slice
    NXC = 8                  # x_a DMA/amax chunks (2 k-tiles = 1MB each)
    NWC = 6                  # w-rest DMA chunks (2 k-tiles = 1MB each)

    with tile.TileContext(nc) as tc, ExitStack() as ctx:
        pool = ctx.enter_context(tc.tile_pool(name="pool", bufs=1))
        xbpool = ctx.enter_context(tc.tile_pool(name="xbpool", bufs=2))
        opool = ctx.enter_context(tc.tile_pool(name="opool", bufs=2))
        psum = ctx.enter_context(tc.tile_pool(name="psum", bufs=8, space="PSUM"))
        dram = ctx.enter_context(tc.tile_pool(name="dram", bufs=1, space="DRAM"))

        xa_sb = pool.tile([128, KT, HT], F32)       # 64KB/part
        ws_sb = pool.tile([128, KT, DSH], F32)      # 64KB/part
        xq = pool.tile([128, KT, TSH], F8)          # 32KB/part (a|b halves)
        wq = pool.tile([128, KT, DSH], F8)          # 16KB/part
        bias_bc = pool.tile([128, DSH], F16)        # 2KB/part
        bias_row = pool.tile([1, DSH], F32)

        # ---- sync ring: input stream in 3 sync-gated phases.  DMA engine
        # bandwidth is CHIP-shared across the 8 cores; without gating, the
        # other cores' bulk streams starve this core's amax-critical reads
        # and the collective waits on the slowest peer.
        with nc.named_scope("dma_in"):
            nc.sync.dma_start(bias_row[:], bias_d[:])
            xa_view = xa_d[:].rearrange("p (k c) -> p k c", k=KT)
            ws_view = ws_d[:].rearrange("p (k c) -> p k c", k=KT)
            # P1: amax coverage (w slice + x half), 1-ktile chunks
            for c in range(WSLK):
                nc.sync.dma_start(ws_sb[:, c:c + 1, :], ws_view[:, c:c + 1, :])
            xa_dmas = []
            for c in range(KT):
                xa_dmas.append(nc.sync.dma_start(xa_sb[:, c:c + 1, :],
                                                 xa_view[:, c:c + 1, :]))
            # P2/P3 are emitted below, gated on the collective: releasing
            # them on local-P1-complete lets the fastest core's bulk traffic
            # starve the slowest core's P1 (chip-shared DMA bandwidth), which
            # amplifies skew the collective then waits out.

        # ---- local abs-max on DVE, chasing the arrivals (gpsimd bulk ops
        # are slow Q7 software loops AND poison DVE via shared SBUF ports)
        red = pool.tile([128, WSLK + KT], F32)
        am2 = pool.tile([128, 2], F32)
        with nc.named_scope("amax"):
            for c in range(WSLK):
                nc.vector.tensor_reduce(red[:, c:c + 1], ws_sb[:, c:c + 1, :],
                                        mybir.AxisListType.XY,
                                        mybir.AluOpType.max,
                                        apply_absolute_value=True)
            for c in range(KT):
                nc.vector.tensor_reduce(red[:, WSLK + c:WSLK + c + 1],
                                        xa_sb[:, c:c + 1, :],
                                        mybir.AxisListType.XY,
                                        mybir.AluOpType.max,
                                        apply_absolute_value=True)
            nc.vector.tensor_reduce(am2[:, 1:2], red[:, 0:WSLK],
                                    mybir.AxisListType.X, mybir.AluOpType.max)
            nc.vector.tensor_reduce(am2[:, 0:1], red[:, WSLK:],
                                    mybir.AxisListType.X, mybir.AluOpType.max)

        # ---- global amax: AllGather [1,2]->[8,2] (small DMAs on ACT ring)
        with nc.named_scope("cc_amax"):
            amg_loc = pool.tile([128, 2], F32)
            nc.gpsimd.partition_all_reduce(amg_loc[:], am2[:], 128,
                                           bass_isa.ReduceOp.max)
            cc_in = dram.tile([1, 2], F32)
            cc_out = dram.tile([8, 2], F32)
            ag8 = pool.tile([8, 2], F32)
            nc.scalar.dma_start(cc_in[:], amg_loc[0:1, :])
            cc_inst = nc.gpsimd.collective_compute(
                "AllGather", mybir.AluOpType.bypass,
                replica_groups=[list(range(N_CORES))],
                ins=[cc_in[:].opt()], outs=[cc_out[:].opt()])
            nc.scalar.dma_start(ag8[:], cc_out[:])
            agr = pool.tile([8, 2], F32)
            nc.gpsimd.partition_all_reduce(agr[:], ag8[:], 8,
                                           bass_isa.ReduceOp.max)
            agb = pool.tile([128, 2], F32)
            nc.gpsimd.partition_broadcast(agb[:], agr[0:1, :])

        # ---- P2 (w rest) then P3 (x other half): with the early barrier
        # absorbing start skew, all cores finish P1 together, so gating on
        # local-P1 no longer lets fast cores starve the laggard's P1
        with nc.named_scope("dma_late"):
            p2 = []
            for c in range(NWC):
                k0 = WSLK + 2 * c
                d = nc.sync.dma_start(ws_sb[:, k0:k0 + 2, :],
                                      ws_view[:, k0:k0 + 2, :])
                add_dep_helper(d.ins, cc_inst.ins, sync=True,
                               reason="P2 after the collective barrier: "
                               "releasing bulk DMA early lets fast cores "
                               "starve the laggard's P1 (chip-shared DMA), "
                               "delaying the collective more than the "
                               "prefetch saves (measured +7us)")
                p2.append(d)
            xb_tiles = []
            for b in range(NB):
                xbt = xbpool.tile([128, KT, 128], F32, name="xbt")
                d = nc.sync.dma_start(xbt[:], xb_d[b * 128:(b + 1) * 128, :])
                add_dep_helper(d.ins, p2[-1].ins, sync=True,
                               reason="P3 after P2")
                xb_tiles.append(xbt)

        # ---- scales: rx = 1/(2 RC ax), rw = 1/(2 RC max(ax,aw)),
        # sc4 = 4 sx sw = (2 RC ax)(2 RC max) -- 5 DVE ops total
        scal = pool.tile([128, 8], F32)  # [d2x, d2w, rx, rw, sc4, ...]
        with nc.named_scope("scales"):
            t01 = pool.tile([128, 2], F32)
            nc.vector.tensor_tensor(t01[:, 1:2], agb[:, 0:1], agb[:, 1:2],
                                    mybir.AluOpType.max)
            nc.vector.tensor_copy(t01[:, 0:1], agb[:, 0:1])
            nc.vector.tensor_scalar(scal[:, 0:2], t01[:], 2.0 * RC, None,
                                    mybir.AluOpType.mult)
            nc.vector.reciprocal(scal[:, 2:4], scal[:, 0:2])    # rx, rw
            nc.vector.tensor_tensor(scal[:, 4:5], scal[:, 0:1],
                                    scal[:, 1:2], mybir.AluOpType.mult)

        # bias broadcast (gpsimd, cheap, before xb quants)
        bias16 = pool.tile([1, DSH], F16)
        nc.vector.tensor_copy(bias16[:], bias_row[:])
        nc.gpsimd.partition_broadcast(bias_bc[:], bias16[:])

        # ---- PE warm-up: junk matmuls gated on P1-DMA-complete keep the PE
        # clock boosted (HAM) through the collective window so real matmuls
        # start at full rate instead of ramping for 3us
        warm_l = pool.tile([128, 2, 128], F8)
        warm_r = pool.tile([128, 2, 512], F8)
        nc.vector.memset(warm_l[:], 0.0)
        nc.vector.memset(warm_r[:], 0.0)
        warm_ps = psum.tile([128, 512], F32, name="ps")
        for i in range(16):
            w_mm = nc.tensor.matmul(warm_ps[:], warm_l[:], warm_r[:],
                                    start=True, stop=True,
                                    perf_mode=mybir.MatmulPerfMode.DoubleRow)
            if i == 0:
                add_dep_helper(w_mm.ins, xa_dmas[-1].ins, sync=True,
                               reason="warm PE during collective window")

        # ---- quantize (wq, xq_a) per k-chunk; pairs kk 0-4 on DVE
        # (1.1us/chunk), kk 5-7 on ACT (2.1us/chunk) so the PE's k-ordered
        # chase sees pairs at the combined rate
        with nc.named_scope("quant"):
            for c in range(KT // 2):
                if c < 5:
                    nc.vector.tensor_scalar(wq[:, 2 * c:2 * c + 2, :],
                                            ws_sb[:, 2 * c:2 * c + 2, :],
                                            scal[:, 3:4], None,
                                            mybir.AluOpType.mult)
                    nc.vector.tensor_scalar(xq[:, 2 * c:2 * c + 2, 0:HT],
                                            xa_sb[:, 2 * c:2 * c + 2, :],
                                            scal[:, 2:3], None,
                                            mybir.AluOpType.mult)
                else:
                    nc.scalar.activation(
                        wq[:, 2 * c:2 * c + 2, :],
                        ws_sb[:, 2 * c:2 * c + 2, :],
                        bass_rust.ActivationFunctionType.Copy,
                        bias=0.0, scale=scal[:, 3:4])
                    nc.scalar.activation(
                        xq[:, 2 * c:2 * c + 2, 0:HT],
                        xa_sb[:, 2 * c:2 * c + 2, :],
                        bass_rust.ActivationFunctionType.Copy,
                        bias=0.0, scale=scal[:, 2:3])

        # ---- matmuls: blocked quant-chasing schedule + per-block tail
        def mm_group(m, ptiles, xcols):
            for kk in range(KT // 2):
                for n in range(GN):
                    nc.tensor.matmul(
                        ptiles[n][:],
                        xq[:, 2 * kk:2 * kk + 2, xcols],
                        wq[:, 2 * kk:2 * kk + 2, n * 512:(n + 1) * 512],
                        start=(kk == 0), stop=(kk == KT // 2 - 1),
                        perf_mode=mybir.MatmulPerfMode.DoubleRow)

        def drain(m, ptiles):
            osb = opool.tile([128, DSH], F16, name="osb")
            for n in range(GN):
                nc.vector.scalar_tensor_tensor(
                    osb[:, n * 512:(n + 1) * 512], ptiles[n][:],
                    scal[:, 4:5], bias_bc[:, n * 512:(n + 1) * 512],
                    mybir.AluOpType.mult, mybir.AluOpType.add)
            nc.scalar.dma_start(out_d[m * 128:(m + 1) * 128, :], osb[:])

        with nc.named_scope("mm"):
            # m0..7 (x_a half): two 4-m-group blocks, k-chunk-major so the
            # PE chases the wq/xq_a quant streams chunk by chunk
            for blk in range(2):
                ptiles = [[psum.tile([128, 512], F32, name="ps")
                           for _ in range(GN)] for _ in range(4)]
                for kk in range(KT // 2):
                    for mi in range(4):
                        m = blk * 4 + mi
                        for n in range(GN):
                            nc.tensor.matmul(
                                ptiles[mi][n][:],
                                xq[:, 2 * kk:2 * kk + 2,
                                   m * 128:(m + 1) * 128],
                                wq[:, 2 * kk:2 * kk + 2,
                                   n * 512:(n + 1) * 512],
                                start=(kk == 0), stop=(kk == KT // 2 - 1),
                                perf_mode=mybir.MatmulPerfMode.DoubleRow)
                for mi in range(4):
                    drain(blk * 4 + mi, ptiles[mi])
            # m8..15: each gated only on its own arriving x_b block
            for b in range(NB):
                nc.scalar.activation(
                    xq[:, :, HT + b * 128:HT + (b + 1) * 128],
                    xb_tiles[b][:],
                    bass_rust.ActivationFunctionType.Copy,
                    bias=0.0, scale=scal[:, 2:3])
                m = 8 + b
                ptiles = [psum.tile([128, 512], F32, name="ps")
                          for _ in range(GN)]
                mm_group(m, ptiles, slice(m * 128, (m + 1) * 128))
                drain(m, ptiles)

    nc.compile()
    _built = nc
    return nc


def _pmaj(a, ktp):
    """[DIN, C] -> [128, KT*C] partition-major with k-tile permutation."""
    return np.ascontiguousarray(
        a.reshape(KT, 128, -1)[ktp].transpose(1, 0, 2).reshape(128, -1))


def stage_inputs(x, weight, bias):
    x = np.asarray(x, dtype=np.float32)
    weight = np.asarray(weight, dtype=np.float32)
    bias = np.asarray(bias, dtype=np.float32)
    x2 = x.reshape(TOK, DIN)
    wt = weight.T                                   # [DIN, DOUT]
    in_maps = []
    for i in range(N_CORES):
        t, n = divmod(i, GN)
        ktp = np.r_[np.arange(4 * t, 4 * t + 4),
                    [k for k in range(KT) if not 4 * t <= k < 4 * t + 4]]
        xs = x2[2048 * t:2048 * (t + 1)].T          # [DIN, 2048]
        xa = xs[:, HT * n:HT * (n + 1)]             # this core's amax half
        xb = xs[:, HT * (1 - n):HT * (2 - n)]
        xbt = xb.reshape(KT, 128, NB, 128)[ktp]     # [k,p,b,c]
        in_maps.append({
            "xa": _pmaj(xa, ktp),
            "xb": np.ascontiguousarray(
                xbt.transpose(2, 1, 0, 3).reshape(NB * 128, KT * 128)),
            "ws": _pmaj(wt[:, DSH * n:DSH * (n + 1)], ktp),
            "bias": np.ascontiguousarray(bias[DSH * n:DSH * (n + 1)]
                                         ).reshape(1, DSH),
        })
    return in_maps


def assemble(results):
    full = np.empty((TOK, DOUT), np.float32)
    for i, r in enumerate(results):
        t, n = divmod(i, GN)
        o = np.asarray(r["out"]).astype(np.float32)
        full[2048 * t + HT * n:2048 * t + HT * (n + 1),
             DSH * n:DSH * (n + 1)] = o[0:HT]
        full[2048 * t + HT * (1 - n):2048 * t + HT * (2 - n),
             DSH * n:DSH * (n + 1)] = o[HT:TSH]
    return full.reshape(B, S, DOUT)


def kernel(x, weight, bias):
    in_maps = stage_inputs(x, weight, bias)
    nc = _build()
    br = run_bass_kernel_spmd(nc, in_maps, list(range(N_CORES)))
    return assemble(br.results)
